# revision 12
# baseline (speedup 1.0000x reference)
"""Trainium2 Bass kernel for the DOMINANT-style GCN autoencoder.

kernel(**inputs) takes the FULL inputs (x [N,256], edge_index [2,E], weights)
and returns (A_hat [N,N], X_hat [N,256]) exactly like the reference.

Strategy (8 NeuronCores, SPMD single launch):
  - Nodes sharded by row range across the 8 cores (2048 nodes/core), with a
    per-shard degree-sort permutation so gather batches pad tightly.
  - gcn(z,W,b) == relu(P(z) @ W + b)   with   P(u) = dinv*(A@(u*dinv) + u*dinv)
    so conv3 (Wa1) and conv5 (Ws1) share one aggregation: 4 aggregations total.
  - Each aggregation: scale rows by dinv -> AllGather [N,128] into shared DRAM
    -> dma_gather of edge sources (dst-bucketed, padded per 128-node batch)
    -> strided tensor_reduce segment sums -> + self term -> *dinv.
  - A_hat = s @ s.T: s all-gathered, regathered into true node order,
    PE-transposed into sT [128, N] held in SBUF; row-shard matmuls stream
    [128,512] PSUM tiles through DVE/ACT copies out to HBM (128 MiB/core).
"""

import os
import sys

for _p in ("/opt/trn_rl_repo", "/root/.axon_site/_ro/trn_rl_repo"):
    if os.path.isdir(_p) and _p not in sys.path:
        sys.path.insert(0, _p)

import numpy as np

import concourse.bacc as bacc
import concourse.bass as bass
import concourse.mybir as mybir
import concourse.tile as tile
from concourse.bass_utils import run_bass_kernel_spmd
from concourse.library_config import mlp

F32 = mybir.dt.float32
I16 = mybir.dt.int16

NCORES = 8
N = 16384
E = 524288
NFEAT = 256
NHID = 128
P = 128  # partitions


# --------------------------------------------------------------------------
# Host-side preprocessing
# --------------------------------------------------------------------------

def _preprocess(edge_index, n=N):
    """Degree/permutation/gather-index computation. O(N+E) host work."""
    src = np.asarray(edge_index[0], dtype=np.int64)
    dst = np.asarray(edge_index[1], dtype=np.int64)
    s = n // NCORES
    nb = s // P  # batches of 128 nodes per shard

    cnt = np.bincount(dst, minlength=n)  # edge in-degree (no self loop)
    deg = cnt.astype(np.float32) + 1.0
    dinv = (1.0 / np.sqrt(deg)).astype(np.float32)

    # CSR of edges sorted by dst
    order = np.argsort(dst, kind="stable")
    src_s = src[order]
    starts = np.zeros(n + 1, np.int64)
    np.cumsum(cnt, out=starts[1:])

    # per-shard degree sort -> row2node / node2row
    row2node = np.empty(n, np.int64)
    node2row = np.empty(n, np.int64)
    for c in range(NCORES):
        nodes = np.arange(c * s, (c + 1) * s)
        o = np.argsort(cnt[nodes], kind="stable")
        rn = nodes[o]
        row2node[c * s:(c + 1) * s] = rn
        node2row[rn] = np.arange(c * s, (c + 1) * s)

    # uniform (across cores) padded batch lengths
    Ls = []
    for b in range(nb):
        m = 1
        for c in range(NCORES):
            rows = row2node[c * s + b * P: c * s + (b + 1) * P]
            m = max(m, int(cnt[rows].max()))
        Ls.append(m)

    zrow = n  # index of the zeros row appended to every gather source

    def wrap16(flat):
        # device layout: index i lives at [partition i%16, col i//16],
        # replicated across the 8 Q7 core groups -> [128, len/16]
        w = flat.reshape(-1, 16).T
        return np.tile(w, (8, 1)).astype(np.int16)

    idx_inputs = []
    for c in range(NCORES):
        blocks = []
        for b in range(nb):
            L = Ls[b]
            arr = np.full((L, P), zrow, np.int64)  # [col, partition]
            rows = row2node[c * s + b * P: c * s + (b + 1) * P]
            for p in range(P):
                v = rows[p]
                s0, s1 = starts[v], starts[v + 1]
                if s1 > s0:
                    arr[: s1 - s0, p] = node2row[src_s[s0:s1]]
            blocks.append(wrap16(arr.reshape(-1)))
        idx_inputs.append(np.concatenate(blocks, axis=1))

    sidx = wrap16(node2row.copy())  # true node order -> permuted global row

    dinv_sb = []
    for c in range(NCORES):
        d = dinv[row2node[c * s:(c + 1) * s]].reshape(nb, P).T  # [128, nb]
        dinv_sb.append(np.ascontiguousarray(d.astype(np.float32)))

    return dict(
        s=s, nb=nb, Ls=Ls, row2node=row2node, node2row=node2row,
        idx_inputs=idx_inputs, sidx=sidx, dinv_sb=dinv_sb,
    )


# --------------------------------------------------------------------------
# Device program
# --------------------------------------------------------------------------

def build_program(n, Ls, f32r_ahat=False, stop_stage=None, repeat=1):
    """Build the SPMD Bass program. All cores run the same program; per-core
    behaviour differs only through input data (idx tables, x shard, dinv).
    stop_stage truncates the program early (hang bisection)."""
    s = n // NCORES
    nb = s // P
    njc = n // 512  # A_hat column chunks of 512
    nc = bacc.Bacc("TRN2", target_bir_lowering=False, debug=False,
                   num_devices=NCORES)
    rg = [list(range(NCORES))]
    idx_cols = sum(8 * L for L in Ls)

    # ---- external I/O ----
    ein = lambda name, shape, dt=F32: nc.dram_tensor(name, shape, dt,
                                                     kind="ExternalInput")
    xT0 = ein("xT0", [P, s])
    xT1 = ein("xT1", [P, s])
    We1d = ein("We1", [NFEAT, NHID])
    We2d = ein("We2", [NHID, NHID])
    Wa1d = ein("Wa1", [NHID, NHID])
    Ws1d = ein("Ws1", [NHID, NHID])
    Wa2d = ein("Wa2", [NHID, NFEAT])
    be1d = ein("be1", [P, 1])
    be2d = ein("be2", [P, 1])
    ba1d = ein("ba1", [P, 1])
    bs1d = ein("bs1", [P, 1])
    ba2d = ein("ba2", [P, 2])
    dinvd = ein("dinv", [P, nb])
    idxd = ein("idx", [P, idx_cols], I16)
    sidxd = ein("sidx", [P, n // 16], I16)
    identd = ein("ident", [P, P])

    arow = nc.dram_tensor("arow", [s, n], F32, kind="ExternalOutput")
    xhat0 = nc.dram_tensor("xhat0", [P, s], F32, kind="ExternalOutput")
    xhat1 = nc.dram_tensor("xhat1", [P, s], F32, kind="ExternalOutput")

    # ---- internal DRAM ----
    bounce = [nc.dram_tensor(f"bnc{k}", [s, NHID], F32) for k in range(5)]
    agout = [nc.dram_tensor(f"ag{k}", [n + P, NHID], F32, addr_space="Shared")
             for k in range(5)]

    with tile.TileContext(nc) as tc:
        with (
            tc.tile_pool(name="const", bufs=1) as cpool,
            tc.tile_pool(name="fm", bufs=1) as fmpool,
            tc.tile_pool(name="pmm", bufs=4, space="PSUM") as pmm,
            tc.tile_pool(name="ptr", bufs=4, space="PSUM") as ptr,
        ):
            nc.gpsimd.load_library(mlp)

            def load_const(pool, dram, shape, tag, dt=F32, src=None):
                t = pool.tile(shape, dt, tag=tag, name=tag)
                nc.sync.dma_start(t[:], dram[:] if src is None else src)
                return t

            we1a = load_const(cpool, We1d, [P, NHID], "we1a", src=We1d[0:P, :])
            we1b = load_const(cpool, We1d, [P, NHID], "we1b",
                              src=We1d[P:NFEAT, :])
            we2 = load_const(cpool, We2d, [P, NHID], "we2")
            wa1 = load_const(cpool, Wa1d, [P, NHID], "wa1")
            ws1 = load_const(cpool, Ws1d, [P, NHID], "ws1")
            wa2 = load_const(cpool, Wa2d, [P, NFEAT], "wa2")
            be1 = load_const(cpool, be1d, [P, 1], "be1")
            be2 = load_const(cpool, be2d, [P, 1], "be2")
            ba1 = load_const(cpool, ba1d, [P, 1], "ba1")
            bs1 = load_const(cpool, bs1d, [P, 1], "bs1")
            ba2 = load_const(cpool, ba2d, [P, 2], "ba2")
            dinv = load_const(cpool, dinvd, [P, nb], "dinv")
            sidx = load_const(cpool, sidxd, [P, n // 16], "sidx", dt=I16)
            ident = load_const(cpool, identd, [P, P], "ident")

            idx_off = np.zeros(nb, np.int64)
            acc = 0
            for b in range(nb):
                idx_off[b] = acc
                acc += 8 * Ls[b]

            mmw = min(512, s)
            relu = mybir.ActivationFunctionType.Relu

            def mm_chunks(lhsT_list, out_cb):
                """out_cb(chunk_j, psum_tile) for lhsT.T @ rhs over column
                chunks of the feature-major rhs [128, s]."""
                for j in range(s // mmw):
                    ps = pmm.tile([P, mmw], F32, tag="mmps", name="ps")
                    for ki, lt in enumerate(lhsT_list):
                        nc.tensor.matmul(
                            ps[:], lt[0], lt[1][:, j * mmw:(j + 1) * mmw],
                            start=(ki == 0), stop=(ki == len(lhsT_list) - 1))
                    out_cb(j, ps)

            def new_fm(tag="fmbuf", bufs=4):
                return fmpool.tile([P, s], F32, tag=tag, bufs=bufs, name=tag)

            # ================= conv phase =================
            def conv_phase(vpool, gp):
                xt0 = load_const(vpool, xT0, [P, s], "xt0")
                xt1 = load_const(vpool, xT1, [P, s], "xt1")
                idxs = load_const(vpool, idxd, [P, idx_cols], "idxs", dt=I16)

                # zero the padding rows of the gather sources
                zt = vpool.tile([P, NHID], F32, tag="zero", name="zt")
                nc.vector.memset(zt[:], 0.0)
                for k in range(5):
                    nc.sync.dma_start(agout[k][n:n + P, :], zt[:])

                def to_nm_scaled(fm_tile):
                    """fm [128, s] -> node-major [128,128] tiles * dinv."""
                    tiles = []
                    for b in range(nb):
                        pt = ptr.tile([P, P], F32, tag="trps", name="pt")
                        nc.tensor.transpose(
                            pt[:], fm_tile[:, b * P:(b + 1) * P], ident[:])
                        u = vpool.tile([P, P], F32, tag="unm", bufs=18,
                                       name="u")
                        nc.scalar.activation(
                            u[:], pt[:], mybir.ActivationFunctionType.Copy,
                            scale=dinv[:, b:b + 1])
                        tiles.append(u)
                    return tiles

                def aggregate(k, u_tiles, stop=None):
                    """P(): AllGather scaled rows, gather + segment sums,
                    add self term, scale by dinv -> node-major q tiles."""
                    for b in range(nb):
                        nc.sync.dma_start(bounce[k][b * P:(b + 1) * P, :],
                                          u_tiles[b][:])
                    nc.gpsimd.collective_compute(
                        "AllGather", mybir.AluOpType.bypass, replica_groups=rg,
                        ins=[bounce[k].ap().opt()],
                        outs=[agout[k][0:n, :].opt()])
                    if stop == "ag":
                        t0 = vpool.tile([P, NHID], F32, tag="agchk",
                                        name="t0")
                        nc.sync.dma_start(t0[:], agout[k][0:P, :])
                        nc.sync.dma_start(xhat0[:, 0:NHID], t0[:])
                        return None
                    q_tiles = []
                    for b in range(nb):
                        L = Ls[b]
                        g = gp.tile([P, L, NHID], F32, tag="gath", bufs=2,
                                    name="g")
                        o = int(idx_off[b])
                        # SWDGE ring holds 1024 descriptors; larger gathers
                        # hang the Q7 -> split into <=8-column (1024-idx)
                        # sub-gathers landing in adjacent column ranges.
                        for c0 in range(0, L, 8):
                            cw = min(8, L - c0)
                            nc.gpsimd.dma_gather(
                                g[:, c0:c0 + cw, :], agout[k].ap(),
                                idxs[:, o + 8 * c0:o + 8 * (c0 + cw)],
                                num_idxs=P * cw, num_idxs_reg=P * cw,
                                elem_size=NHID)
                        red = vpool.tile([P, P], F32, tag="red", bufs=4,
                                         name="red")
                        if stop == "gat":
                            nc.vector.tensor_copy(red[:], g[:, 0, :])
                            nc.sync.dma_start(xhat0[:, b * P:(b + 1) * P],
                                              red[:])
                            continue
                        nc.vector.tensor_reduce(
                            red[:], g.rearrange("p c f -> p f c"),
                            axis=mybir.AxisListType.X, op=mybir.AluOpType.add)
                        nc.vector.tensor_tensor(
                            red[:], red[:], u_tiles[b][:],
                            op=mybir.AluOpType.add)
                        q = vpool.tile([P, P], F32, tag="qnm", bufs=18,
                                       name="q")
                        nc.vector.tensor_scalar_mul(q[:], red[:],
                                                    dinv[:, b:b + 1])
                        q_tiles.append(q)
                    if stop == "gat":
                        return None
                    return q_tiles

                def to_fm(q_tiles, func=None, bias=0.0, tag="fmbuf", bufs=4):
                    """node-major -> fm [128, s] via PE transpose, applying
                    func/bias (per-partition == per-feature) on the way."""
                    fm = new_fm(tag, bufs)
                    f = func or mybir.ActivationFunctionType.Copy
                    for b in range(nb):
                        pt = ptr.tile([P, P], F32, tag="trps", name="pt")
                        nc.tensor.transpose(pt[:], q_tiles[b][:], ident[:])
                        if isinstance(bias, float):
                            nc.scalar.activation(
                                fm[:, b * P:(b + 1) * P], pt[:], f)
                        else:
                            nc.scalar.activation(
                                fm[:, b * P:(b + 1) * P], pt[:], f, bias=bias)
                    return fm

                # conv1: h1 = x @ We1 ; z1 = relu(P(h1) + be1)
                h1f = new_fm()
                mm_chunks([(we1a, xt0), (we1b, xt1)],
                          lambda j, ps: nc.scalar.copy(
                              h1f[:, j * mmw:(j + 1) * mmw], ps[:]))
                if stop_stage == "h1":
                    nc.sync.dma_start(xhat0[:, :], h1f[:])
                    return None
                u1t = to_nm_scaled(h1f)
                if stop_stage == "u1":
                    nc.sync.dma_start(xhat0[:, 0:P], u1t[0][:])
                    return None
                q1 = aggregate(0, u1t,
                               stop={"ag1": "ag", "gat1": "gat"}.get(
                                   stop_stage))
                if q1 is None:
                    return None
                if stop_stage == "q1":
                    nc.sync.dma_start(xhat0[:, 0:P], q1[0][:])
                    return None
                z1f = to_fm(q1, func=relu, bias=be1[:, 0:1])
                if stop_stage == "z1":
                    nc.sync.dma_start(xhat0[:, :], z1f[:])
                    return None

                # conv2: h2 = z1 @ We2 ; z2 = relu(P(h2) + be2)
                h2f = new_fm()
                mm_chunks([(we2, z1f)],
                          lambda j, ps: nc.scalar.copy(
                              h2f[:, j * mmw:(j + 1) * mmw], ps[:]))
                q2 = aggregate(1, to_nm_scaled(h2f))
                z2f = to_fm(q2, func=relu, bias=be2[:, 0:1])
                if stop_stage == "conv2":
                    nc.sync.dma_start(xhat0[:, :], z2f[:])
                    return None

                # shared aggregation for conv3 (Wa1) and conv5 (Ws1)
                q3 = aggregate(2, to_nm_scaled(z2f))
                q3f = to_fm(q3)

                # a = relu(q3 @ Wa1 + ba1); s = relu(q3 @ Ws1 + bs1)
                af = new_fm()
                mm_chunks([(wa1, q3f)],
                          lambda j, ps: nc.scalar.activation(
                              af[:, j * mmw:(j + 1) * mmw], ps[:], relu,
                              bias=ba1[:, 0:1]))
                sf = new_fm(tag="sf", bufs=1)
                mm_chunks([(ws1, q3f)],
                          lambda j, ps: nc.scalar.activation(
                              sf[:, j * mmw:(j + 1) * mmw], ps[:], relu,
                              bias=bs1[:, 0:1]))
                if stop_stage == "conv3":
                    nc.sync.dma_start(xhat0[:, :], af[:])
                    nc.sync.dma_start(xhat1[:, :], sf[:])
                    return None

                # conv4: X_hat = relu(P(a) @ Wa2 + ba2)
                q4 = aggregate(3, to_nm_scaled(af))
                q4f = to_fm(q4)
                for h, xdram in enumerate((xhat0, xhat1)):
                    xf = new_fm()
                    mm_chunks([(wa2[:, h * P:(h + 1) * P], q4f)],
                              lambda j, ps, xf=xf, h=h: nc.scalar.activation(
                                  xf[:, j * mmw:(j + 1) * mmw], ps[:], relu,
                                  bias=ba2[:, h:h + 1]))
                    nc.sync.dma_start(xdram[:, :], xf[:])

                # conv5 output s: to DRAM (permuted row order) + AllGather
                for b in range(nb):
                    pt = ptr.tile([P, P], F32, tag="trps", name="pt")
                    nc.tensor.transpose(pt[:], sf[:, b * P:(b + 1) * P],
                                        ident[:])
                    snm = vpool.tile([P, P], F32, tag="snm", bufs=4,
                                     name="snm")
                    nc.scalar.copy(snm[:], pt[:])
                    nc.sync.dma_start(bounce[4][b * P:(b + 1) * P, :],
                                      snm[:])
                nc.gpsimd.collective_compute(
                    "AllGather", mybir.AluOpType.bypass, replica_groups=rg,
                    ins=[bounce[4].ap().opt()],
                    outs=[agout[4][0:n, :].opt()])
                return sf

            for _rep in range(repeat):
              with (
                tc.tile_pool(name=f"convp{_rep}", bufs=1) as vpool,
                tc.tile_pool(name=f"gat{_rep}", bufs=1) as gp,
              ):
                sf = conv_phase(vpool, gp)

              # ================= A_hat phase =================
              if sf is not None and stop_stage != "noahat":
                with tc.tile_pool(name=f"ahat{_rep}", bufs=1) as apool:
                    # regather s into true node order; transpose -> sT [128,n]
                    sT = apool.tile([P, n], F32, tag="sT", name="sT")
                    rch = min(1024, n)
                    for r in range(n // rch):
                        rb = apool.tile([P, rch // P, NHID], F32, tag="rgath",
                                        bufs=2, name="rb")
                        nc.gpsimd.dma_gather(
                            rb[:], agout[4].ap(),
                            sidx[:, r * (rch // 16):(r + 1) * (rch // 16)],
                            num_idxs=rch, num_idxs_reg=rch, elem_size=NHID)
                        for cth in range(rch // P):
                            jcol = r * (rch // P) + cth
                            pt = ptr.tile([P, P], F32, tag="trps", name="pt")
                            nc.tensor.transpose(pt[:], rb[:, cth, :],
                                                ident[:])
                            nc.scalar.copy(sT[:, jcol * P:(jcol + 1) * P],
                                           pt[:])

                    # arow = s_shard @ s.T
                    if f32r_ahat:
                        sfm_mm = sf.bitcast(mybir.dt.float32r)
                        sT_mm = sT.bitcast(mybir.dt.float32r)
                    else:
                        sfm_mm, sT_mm = sf, sT
                    for i in range(nb):
                        lhsT = sfm_mm[:, i * P:(i + 1) * P]
                        for jo in range(njc // 4):
                            stg = apool.tile([P, 2048], F32, tag="astg",
                                             bufs=4, name="stg")
                            for ji in range(4):
                                j = jo * 4 + ji
                                ps = pmm.tile([P, 512], F32, tag="mmps",
                                              name="ps")
                                nc.tensor.matmul(
                                    ps[:], lhsT,
                                    sT_mm[:, j * 512:(j + 1) * 512],
                                    start=True, stop=True)
                                dst = stg[:, ji * 512:(ji + 1) * 512]
                                if ji % 2 == 0:
                                    nc.vector.tensor_copy(dst, ps[:])
                                else:
                                    nc.scalar.copy(dst, ps[:])
                            nc.sync.dma_start(
                                arow[i * P:(i + 1) * P,
                                     jo * 2048:(jo + 1) * 2048],
                                stg[:])

    nc.compile()
    return nc


# --------------------------------------------------------------------------
# Host entry point
# --------------------------------------------------------------------------

_PROGRAM_CACHE = {}


def _get_program(n, Ls, f32r_ahat=False):
    key = (n, tuple(Ls), f32r_ahat)
    if key not in _PROGRAM_CACHE:
        _PROGRAM_CACHE[key] = build_program(n, Ls, f32r_ahat)
    return _PROGRAM_CACHE[key]


def make_in_maps(x, edge_index, We1, be1, We2, be2, Wa1, ba1, Wa2, ba2,
                 Ws1, bs1, pre=None, n=N):
    s = n // NCORES
    if pre is None:
        pre = _preprocess(edge_index, n)
    x = np.asarray(x, np.float32)
    in_maps = []
    shared = {
        "We1": np.asarray(We1, np.float32),
        "We2": np.asarray(We2, np.float32),
        "Wa1": np.asarray(Wa1, np.float32),
        "Ws1": np.asarray(Ws1, np.float32),
        "Wa2": np.asarray(Wa2, np.float32),
        "be1": np.asarray(be1, np.float32).reshape(P, 1),
        "be2": np.asarray(be2, np.float32).reshape(P, 1),
        "ba1": np.asarray(ba1, np.float32).reshape(P, 1),
        "bs1": np.asarray(bs1, np.float32).reshape(P, 1),
        "ba2": np.ascontiguousarray(
            np.asarray(ba2, np.float32).reshape(2, P).T),
        "sidx": pre["sidx"],
        "ident": np.eye(P, dtype=np.float32),
    }
    for c in range(NCORES):
        rows = pre["row2node"][c * s:(c + 1) * s]
        xs = np.ascontiguousarray(x[rows].T)  # [NFEAT, s]
        m = dict(shared)
        m["xT0"] = np.ascontiguousarray(xs[0:P])
        m["xT1"] = np.ascontiguousarray(xs[P:NFEAT])
        m["idx"] = pre["idx_inputs"][c]
        m["dinv"] = pre["dinv_sb"][c]
        in_maps.append(m)
    return in_maps, pre


def assemble(results, pre, n=N):
    s = n // NCORES
    A = np.empty((n, n), np.float32)
    X = np.empty((n, NFEAT), np.float32)
    for c in range(NCORES):
        rows = pre["row2node"][c * s:(c + 1) * s]
        A[rows, :] = results[c]["arow"]
        xh = np.concatenate([results[c]["xhat0"], results[c]["xhat1"]],
                            axis=0)
        X[rows, :] = xh.T
    return A, X


def kernel(x, edge_index, We1, be1, We2, be2, Wa1, ba1, Wa2, ba2, Ws1, bs1,
           trace=False):
    pre = _preprocess(edge_index, N)
    in_maps, pre = make_in_maps(x, edge_index, We1, be1, We2, be2, Wa1, ba1,
                                Wa2, ba2, Ws1, bs1, pre=pre, n=N)
    nc = _get_program(N, pre["Ls"])
    br = run_bass_kernel_spmd(nc, in_maps, list(range(NCORES)), trace=trace)
    A, X = assemble(br.results, pre, N)
    if trace:
        kernel.last_result = br
    return (A, X)


# revision 15
# speedup vs baseline: 268.6073x; 268.6073x over previous
"""Trainium2 Bass kernel for the DOMINANT-style GCN autoencoder.

kernel(**inputs) takes the FULL inputs (x [N,256], edge_index [2,E], weights)
and returns (A_hat [N,N], X_hat [N,256]) exactly like the reference.

Strategy (8 NeuronCores, SPMD single launch):
  - Nodes sharded by row range across the 8 cores (2048 nodes/core), with a
    per-shard degree-sort permutation so gather batches pad tightly.
  - gcn(z,W,b) == relu(P(z) @ W + b)   with   P(u) = dinv*(A@(u*dinv) + u*dinv)
    so conv3 (Wa1) and conv5 (Ws1) share one aggregation: 4 aggregations total.
  - Each aggregation: scale rows by dinv -> AllGather [N,128] into shared DRAM
    -> dma_gather of edge sources (dst-bucketed, padded per 128-node batch)
    -> strided tensor_reduce segment sums -> + self term -> *dinv.
  - A_hat = s @ s.T: s all-gathered, regathered into true node order,
    PE-transposed into sT [128, N] held in SBUF; row-shard matmuls stream
    [128,512] PSUM tiles through DVE/ACT copies out to HBM (128 MiB/core).
"""

import os
import sys

for _p in ("/opt/trn_rl_repo", "/root/.axon_site/_ro/trn_rl_repo"):
    if os.path.isdir(_p) and _p not in sys.path:
        sys.path.insert(0, _p)

import numpy as np

import concourse.bacc as bacc
import concourse.bass as bass
import concourse.mybir as mybir
import concourse.tile as tile
from concourse.bass_utils import run_bass_kernel_spmd
from concourse.library_config import mlp

F32 = mybir.dt.float32
I16 = mybir.dt.int16

NCORES = 8
N = 16384
E = 524288
NFEAT = 256
NHID = 128
P = 128  # partitions


# --------------------------------------------------------------------------
# Host-side preprocessing
# --------------------------------------------------------------------------

def _preprocess(edge_index, n=N):
    """Degree/permutation/gather-index computation. O(N+E) host work."""
    src = np.asarray(edge_index[0], dtype=np.int64)
    dst = np.asarray(edge_index[1], dtype=np.int64)
    s = n // NCORES
    nb = s // P  # batches of 128 nodes per shard

    cnt = np.bincount(dst, minlength=n)  # edge in-degree (no self loop)
    deg = cnt.astype(np.float32) + 1.0
    dinv = (1.0 / np.sqrt(deg)).astype(np.float32)

    # CSR of edges sorted by dst
    order = np.argsort(dst, kind="stable")
    src_s = src[order]
    starts = np.zeros(n + 1, np.int64)
    np.cumsum(cnt, out=starts[1:])

    # per-shard degree sort -> row2node / node2row
    row2node = np.empty(n, np.int64)
    node2row = np.empty(n, np.int64)
    for c in range(NCORES):
        nodes = np.arange(c * s, (c + 1) * s)
        o = np.argsort(cnt[nodes], kind="stable")
        rn = nodes[o]
        row2node[c * s:(c + 1) * s] = rn
        node2row[rn] = np.arange(c * s, (c + 1) * s)

    # uniform (across cores) padded batch lengths
    Ls = []
    for b in range(nb):
        m = 1
        for c in range(NCORES):
            rows = row2node[c * s + b * P: c * s + (b + 1) * P]
            m = max(m, int(cnt[rows].max()))
        Ls.append(m)

    zrow = n  # index of the zeros row appended to every gather source

    def wrap16(flat):
        # device layout: index i lives at [partition i%16, col i//16],
        # replicated across the 8 Q7 core groups -> [128, len/16]
        w = flat.reshape(-1, 16).T
        return np.tile(w, (8, 1)).astype(np.int16)

    idx_inputs = []
    for c in range(NCORES):
        blocks = []
        for b in range(nb):
            L = Ls[b]
            arr = np.full((L, P), zrow, np.int64)  # [col, partition]
            rows = row2node[c * s + b * P: c * s + (b + 1) * P]
            for p in range(P):
                v = rows[p]
                s0, s1 = starts[v], starts[v + 1]
                if s1 > s0:
                    arr[: s1 - s0, p] = node2row[src_s[s0:s1]]
            blocks.append(wrap16(arr.reshape(-1)))
        idx_inputs.append(np.concatenate(blocks, axis=1))

    sidx = wrap16(node2row.copy())  # true node order -> permuted global row

    dinv_sb = []
    for c in range(NCORES):
        d = dinv[row2node[c * s:(c + 1) * s]].reshape(nb, P).T  # [128, nb]
        dinv_sb.append(np.ascontiguousarray(d.astype(np.float32)))

    return dict(
        s=s, nb=nb, Ls=Ls, row2node=row2node, node2row=node2row,
        idx_inputs=idx_inputs, sidx=sidx, dinv_sb=dinv_sb,
    )


# --------------------------------------------------------------------------
# Device program
# --------------------------------------------------------------------------

def build_program(n, Ls, f32r_ahat=False, stop_stage=None, repeat=1):
    """Build the SPMD Bass program. All cores run the same program; per-core
    behaviour differs only through input data (idx tables, x shard, dinv).
    stop_stage truncates the program early (hang bisection)."""
    s = n // NCORES
    nb = s // P
    njc = n // 512  # A_hat column chunks of 512
    nc = bacc.Bacc("TRN2", target_bir_lowering=False, debug=False,
                   num_devices=NCORES)
    rg = [list(range(NCORES))]
    idx_cols = sum(8 * L for L in Ls)

    # ---- external I/O ----
    ein = lambda name, shape, dt=F32: nc.dram_tensor(name, shape, dt,
                                                     kind="ExternalInput")
    xT0 = ein("xT0", [P, s])
    xT1 = ein("xT1", [P, s])
    We1d = ein("We1", [NFEAT, NHID])
    We2d = ein("We2", [NHID, NHID])
    Wa1d = ein("Wa1", [NHID, NHID])
    Ws1d = ein("Ws1", [NHID, NHID])
    Wa2d = ein("Wa2", [NHID, NFEAT])
    be1d = ein("be1", [P, 1])
    be2d = ein("be2", [P, 1])
    ba1d = ein("ba1", [P, 1])
    bs1d = ein("bs1", [P, 1])
    ba2d = ein("ba2", [P, 2])
    dinvd = ein("dinv", [P, nb])
    idxd = ein("idx", [P, idx_cols], I16)
    sidxd = ein("sidx", [P, n // 16], I16)
    identd = ein("ident", [P, P])

    arow = nc.dram_tensor("arow", [s, n], F32, kind="ExternalOutput")
    xhat0 = nc.dram_tensor("xhat0", [P, s], F32, kind="ExternalOutput")
    xhat1 = nc.dram_tensor("xhat1", [P, s], F32, kind="ExternalOutput")

    # ---- internal DRAM ----
    bounce = [nc.dram_tensor(f"bnc{k}", [s, NHID], F32) for k in range(5)]
    agout = [nc.dram_tensor(f"ag{k}", [n + P, NHID], F32, addr_space="Shared")
             for k in range(5)]

    tc_trace = bool(os.environ.get('KERNEL_TRACE_SIM'))
    with tile.TileContext(nc, trace_sim=tc_trace) as tc:
        with (
            tc.tile_pool(name="const", bufs=1) as cpool,
            tc.tile_pool(name="fm", bufs=1) as fmpool,
            tc.tile_pool(name="pmm", bufs=4, space="PSUM") as pmm,
            tc.tile_pool(name="ptr", bufs=4, space="PSUM") as ptr,
        ):
            nc.gpsimd.load_library(mlp)

            def load_const(pool, dram, shape, tag, dt=F32, src=None):
                t = pool.tile(shape, dt, tag=tag, name=tag)
                nc.sync.dma_start(t[:], dram[:] if src is None else src)
                return t

            we1a = load_const(cpool, We1d, [P, NHID], "we1a", src=We1d[0:P, :])
            we1b = load_const(cpool, We1d, [P, NHID], "we1b",
                              src=We1d[P:NFEAT, :])
            we2 = load_const(cpool, We2d, [P, NHID], "we2")
            wa1 = load_const(cpool, Wa1d, [P, NHID], "wa1")
            ws1 = load_const(cpool, Ws1d, [P, NHID], "ws1")
            wa2 = load_const(cpool, Wa2d, [P, NFEAT], "wa2")
            be1 = load_const(cpool, be1d, [P, 1], "be1")
            be2 = load_const(cpool, be2d, [P, 1], "be2")
            ba1 = load_const(cpool, ba1d, [P, 1], "ba1")
            bs1 = load_const(cpool, bs1d, [P, 1], "bs1")
            ba2 = load_const(cpool, ba2d, [P, 2], "ba2")
            dinv = load_const(cpool, dinvd, [P, nb], "dinv")
            sidx = load_const(cpool, sidxd, [P, n // 16], "sidx", dt=I16)
            ident = load_const(cpool, identd, [P, P], "ident")

            idx_off = np.zeros(nb, np.int64)
            acc = 0
            for b in range(nb):
                idx_off[b] = acc
                acc += 8 * Ls[b]

            mmw = min(512, s)
            relu = mybir.ActivationFunctionType.Relu

            def mm_chunks(lhsT_list, out_cb):
                """out_cb(chunk_j, psum_tile) for lhsT.T @ rhs over column
                chunks of the feature-major rhs [128, s]."""
                for j in range(s // mmw):
                    ps = pmm.tile([P, mmw], F32, tag="mmps", name="ps")
                    for ki, lt in enumerate(lhsT_list):
                        nc.tensor.matmul(
                            ps[:], lt[0], lt[1][:, j * mmw:(j + 1) * mmw],
                            start=(ki == 0), stop=(ki == len(lhsT_list) - 1))
                    out_cb(j, ps)

            def new_fm(tag="fmbuf", bufs=4):
                return fmpool.tile([P, s], F32, tag=tag, bufs=bufs, name=tag)

            # ================= conv phase =================
            def conv_phase(vpool, gp):
                xt0 = load_const(vpool, xT0, [P, s], "xt0")
                xt1 = load_const(vpool, xT1, [P, s], "xt1")
                idxs = load_const(vpool, idxd, [P, idx_cols], "idxs", dt=I16)

                # zero the padding rows of the gather sources
                zt = vpool.tile([P, NHID], F32, tag="zero", name="zt")
                nc.vector.memset(zt[:], 0.0)
                for k in range(5):
                    nc.sync.dma_start(agout[k][n:n + P, :], zt[:])

                def to_nm_scaled(fm_tile):
                    """fm [128, s] -> node-major [128,128] tiles * dinv."""
                    tiles = []
                    for b in range(nb):
                        pt = ptr.tile([P, P], F32, tag="trps", name="pt")
                        nc.tensor.transpose(
                            pt[:], fm_tile[:, b * P:(b + 1) * P], ident[:])
                        u = vpool.tile([P, P], F32, tag="unm", bufs=18,
                                       name="u")
                        nc.scalar.activation(
                            u[:], pt[:], mybir.ActivationFunctionType.Copy,
                            scale=dinv[:, b:b + 1])
                        tiles.append(u)
                    return tiles

                def aggregate(k, u_tiles, stop=None):
                    """P(): AllGather scaled rows, gather + segment sums,
                    add self term, scale by dinv -> node-major q tiles."""
                    for b in range(nb):
                        nc.sync.dma_start(bounce[k][b * P:(b + 1) * P, :],
                                          u_tiles[b][:])
                    nc.gpsimd.collective_compute(
                        "AllGather", mybir.AluOpType.bypass, replica_groups=rg,
                        ins=[bounce[k].ap().opt()],
                        outs=[agout[k][0:n, :].opt()])
                    if stop == "ag":
                        t0 = vpool.tile([P, NHID], F32, tag="agchk",
                                        name="t0")
                        nc.sync.dma_start(t0[:], agout[k][0:P, :])
                        nc.sync.dma_start(xhat0[:, 0:NHID], t0[:])
                        return None
                    q_tiles = []
                    for b in range(nb):
                        L = Ls[b]
                        g = gp.tile([P, L, NHID], F32, tag="gath", bufs=2,
                                    name="g")
                        o = int(idx_off[b])
                        # SWDGE ring holds 1024 descriptors; larger gathers
                        # hang the Q7 -> split into <=8-column (1024-idx)
                        # sub-gathers landing in adjacent column ranges.
                        for c0 in range(0, L, 8):
                            cw = min(8, L - c0)
                            nc.gpsimd.dma_gather(
                                g[:, c0:c0 + cw, :], agout[k].ap(),
                                idxs[:, o + 8 * c0:o + 8 * (c0 + cw)],
                                num_idxs=P * cw, num_idxs_reg=P * cw,
                                elem_size=NHID)
                        red = vpool.tile([P, P], F32, tag="red", bufs=4,
                                         name="red")
                        if stop == "gat":
                            nc.vector.tensor_copy(red[:], g[:, 0, :])
                            nc.sync.dma_start(xhat0[:, b * P:(b + 1) * P],
                                              red[:])
                            continue
                        nc.vector.tensor_reduce(
                            red[:], g.rearrange("p c f -> p f c"),
                            axis=mybir.AxisListType.X, op=mybir.AluOpType.add)
                        nc.vector.tensor_tensor(
                            red[:], red[:], u_tiles[b][:],
                            op=mybir.AluOpType.add)
                        q = vpool.tile([P, P], F32, tag="qnm", bufs=18,
                                       name="q")
                        nc.vector.tensor_scalar_mul(q[:], red[:],
                                                    dinv[:, b:b + 1])
                        q_tiles.append(q)
                    if stop == "gat":
                        return None
                    return q_tiles

                def to_fm(q_tiles, func=None, bias=0.0, tag="fmbuf", bufs=4):
                    """node-major -> fm [128, s] via PE transpose, applying
                    func/bias (per-partition == per-feature) on the way."""
                    fm = new_fm(tag, bufs)
                    f = func or mybir.ActivationFunctionType.Copy
                    for b in range(nb):
                        pt = ptr.tile([P, P], F32, tag="trps", name="pt")
                        nc.tensor.transpose(pt[:], q_tiles[b][:], ident[:])
                        if isinstance(bias, float):
                            nc.scalar.activation(
                                fm[:, b * P:(b + 1) * P], pt[:], f)
                        else:
                            nc.scalar.activation(
                                fm[:, b * P:(b + 1) * P], pt[:], f, bias=bias)
                    return fm

                # conv1: h1 = x @ We1 ; z1 = relu(P(h1) + be1)
                h1f = new_fm()
                mm_chunks([(we1a, xt0), (we1b, xt1)],
                          lambda j, ps: nc.scalar.copy(
                              h1f[:, j * mmw:(j + 1) * mmw], ps[:]))
                if stop_stage == "h1":
                    nc.sync.dma_start(xhat0[:, :], h1f[:])
                    return None
                u1t = to_nm_scaled(h1f)
                if stop_stage == "u1":
                    nc.sync.dma_start(xhat0[:, 0:P], u1t[0][:])
                    return None
                q1 = aggregate(0, u1t,
                               stop={"ag1": "ag", "gat1": "gat"}.get(
                                   stop_stage))
                if q1 is None:
                    return None
                if stop_stage == "q1":
                    nc.sync.dma_start(xhat0[:, 0:P], q1[0][:])
                    return None
                z1f = to_fm(q1, func=relu, bias=be1[:, 0:1])
                if stop_stage == "z1":
                    nc.sync.dma_start(xhat0[:, :], z1f[:])
                    return None

                # conv2: h2 = z1 @ We2 ; z2 = relu(P(h2) + be2)
                h2f = new_fm()
                mm_chunks([(we2, z1f)],
                          lambda j, ps: nc.scalar.copy(
                              h2f[:, j * mmw:(j + 1) * mmw], ps[:]))
                q2 = aggregate(1, to_nm_scaled(h2f))
                z2f = to_fm(q2, func=relu, bias=be2[:, 0:1])
                if stop_stage == "conv2":
                    nc.sync.dma_start(xhat0[:, :], z2f[:])
                    return None

                # shared aggregation for conv3 (Wa1) and conv5 (Ws1)
                q3 = aggregate(2, to_nm_scaled(z2f))
                q3f = to_fm(q3)

                # a = relu(q3 @ Wa1 + ba1); s = relu(q3 @ Ws1 + bs1)
                af = new_fm()
                mm_chunks([(wa1, q3f)],
                          lambda j, ps: nc.scalar.activation(
                              af[:, j * mmw:(j + 1) * mmw], ps[:], relu,
                              bias=ba1[:, 0:1]))
                sf = new_fm(tag="sf", bufs=1)
                mm_chunks([(ws1, q3f)],
                          lambda j, ps: nc.scalar.activation(
                              sf[:, j * mmw:(j + 1) * mmw], ps[:], relu,
                              bias=bs1[:, 0:1]))
                if stop_stage == "conv3":
                    nc.sync.dma_start(xhat0[:, :], af[:])
                    nc.sync.dma_start(xhat1[:, :], sf[:])
                    return None

                # conv4: X_hat = relu(P(a) @ Wa2 + ba2)
                q4 = aggregate(3, to_nm_scaled(af))
                q4f = to_fm(q4)
                for h, xdram in enumerate((xhat0, xhat1)):
                    xf = new_fm()
                    mm_chunks([(wa2[:, h * P:(h + 1) * P], q4f)],
                              lambda j, ps, xf=xf, h=h: nc.scalar.activation(
                                  xf[:, j * mmw:(j + 1) * mmw], ps[:], relu,
                                  bias=ba2[:, h:h + 1]))
                    nc.sync.dma_start(xdram[:, :], xf[:])

                # conv5 output s: to DRAM (permuted row order) + AllGather
                for b in range(nb):
                    pt = ptr.tile([P, P], F32, tag="trps", name="pt")
                    nc.tensor.transpose(pt[:], sf[:, b * P:(b + 1) * P],
                                        ident[:])
                    snm = vpool.tile([P, P], F32, tag="snm", bufs=4,
                                     name="snm")
                    nc.scalar.copy(snm[:], pt[:])
                    nc.sync.dma_start(bounce[4][b * P:(b + 1) * P, :],
                                      snm[:])
                nc.gpsimd.collective_compute(
                    "AllGather", mybir.AluOpType.bypass, replica_groups=rg,
                    ins=[bounce[4].ap().opt()],
                    outs=[agout[4][0:n, :].opt()])
                return sf

            for _rep in range(repeat):
              with (
                tc.tile_pool(name=f"convp{_rep}", bufs=1) as vpool,
                tc.tile_pool(name=f"gat{_rep}", bufs=1) as gp,
              ):
                sf = conv_phase(vpool, gp)

              # ================= A_hat phase =================
              if sf is not None and stop_stage != "noahat":
                with tc.tile_pool(name=f"ahat{_rep}", bufs=1) as apool:
                    # regather s into true node order; transpose -> sT [128,n]
                    sTdt = mybir.dt.float32r if f32r_ahat else F32
                    sT = apool.tile([P, n], sTdt, tag="sT", name="sT")
                    rch = min(1024, n)
                    for r in range(n // rch):
                        rb = apool.tile([P, rch // P, NHID], F32, tag="rgath",
                                        bufs=2, name="rb")
                        nc.gpsimd.dma_gather(
                            rb[:], agout[4].ap(),
                            sidx[:, r * (rch // 16):(r + 1) * (rch // 16)],
                            num_idxs=rch, num_idxs_reg=rch, elem_size=NHID)
                        for cth in range(rch // P):
                            jcol = r * (rch // P) + cth
                            pt = ptr.tile([P, P], F32, tag="trps", name="pt")
                            nc.tensor.transpose(pt[:], rb[:, cth, :],
                                                ident[:])
                            nc.scalar.copy(sT[:, jcol * P:(jcol + 1) * P],
                                           pt[:])

                    # arow = s_shard @ s.T
                    if f32r_ahat:
                        # rounded copy of s for the f32r matmul (the PE f32r
                        # path needs f32r-rounded producers)
                        sfr = apool.tile([P, s], mybir.dt.float32r,
                                         tag="sfr", name="sfr")
                        nc.scalar.copy(sfr[:], sf[:])
                        sfm_mm, sT_mm = sfr, sT
                    else:
                        sfm_mm, sT_mm = sf, sT
                    for i in range(nb):
                        lhsT = sfm_mm[:, i * P:(i + 1) * P]
                        for jo in range(njc // 4):
                            stg = apool.tile([P, 2048], F32, tag="astg",
                                             bufs=4, name="stg")
                            for ji in range(4):
                                j = jo * 4 + ji
                                ps = pmm.tile([P, 512], F32, tag="mmps",
                                              name="ps")
                                nc.tensor.matmul(
                                    ps[:], lhsT,
                                    sT_mm[:, j * 512:(j + 1) * 512],
                                    start=True, stop=True)
                                dst = stg[:, ji * 512:(ji + 1) * 512]
                                if ji % 2 == 0:
                                    nc.vector.tensor_copy(dst, ps[:])
                                else:
                                    nc.scalar.copy(dst, ps[:])
                            nc.sync.dma_start(
                                arow[i * P:(i + 1) * P,
                                     jo * 2048:(jo + 1) * 2048],
                                stg[:])

    nc.compile()
    if tc_trace and getattr(tc, "_perfetto_entries", None):
        # entries: (name, start_ns, end_ns, space, size, addr, tag)
        global LAST_MAKESPAN_NS
        ends = [t[2] for t in tc._perfetto_entries]
        starts = [t[1] for t in tc._perfetto_entries]
        LAST_MAKESPAN_NS = int(max(ends) - min(starts))
        print(f"[cost-model makespan] {LAST_MAKESPAN_NS} ns")
    return nc


# --------------------------------------------------------------------------
# Host entry point
# --------------------------------------------------------------------------

_PROGRAM_CACHE = {}
LAST_MAKESPAN_NS = None


def _get_program(n, Ls, f32r_ahat=False):
    key = (n, tuple(Ls), f32r_ahat)
    if key not in _PROGRAM_CACHE:
        _PROGRAM_CACHE[key] = build_program(n, Ls, f32r_ahat)
    return _PROGRAM_CACHE[key]


def make_in_maps(x, edge_index, We1, be1, We2, be2, Wa1, ba1, Wa2, ba2,
                 Ws1, bs1, pre=None, n=N):
    s = n // NCORES
    if pre is None:
        pre = _preprocess(edge_index, n)
    x = np.asarray(x, np.float32)
    in_maps = []
    shared = {
        "We1": np.asarray(We1, np.float32),
        "We2": np.asarray(We2, np.float32),
        "Wa1": np.asarray(Wa1, np.float32),
        "Ws1": np.asarray(Ws1, np.float32),
        "Wa2": np.asarray(Wa2, np.float32),
        "be1": np.asarray(be1, np.float32).reshape(P, 1),
        "be2": np.asarray(be2, np.float32).reshape(P, 1),
        "ba1": np.asarray(ba1, np.float32).reshape(P, 1),
        "bs1": np.asarray(bs1, np.float32).reshape(P, 1),
        "ba2": np.ascontiguousarray(
            np.asarray(ba2, np.float32).reshape(2, P).T),
        "sidx": pre["sidx"],
        "ident": np.eye(P, dtype=np.float32),
    }
    for c in range(NCORES):
        rows = pre["row2node"][c * s:(c + 1) * s]
        xs = np.ascontiguousarray(x[rows].T)  # [NFEAT, s]
        m = dict(shared)
        m["xT0"] = np.ascontiguousarray(xs[0:P])
        m["xT1"] = np.ascontiguousarray(xs[P:NFEAT])
        m["idx"] = pre["idx_inputs"][c]
        m["dinv"] = pre["dinv_sb"][c]
        in_maps.append(m)
    return in_maps, pre


def assemble(results, pre, n=N):
    s = n // NCORES
    A = np.empty((n, n), np.float32)
    X = np.empty((n, NFEAT), np.float32)
    for c in range(NCORES):
        rows = pre["row2node"][c * s:(c + 1) * s]
        A[rows, :] = results[c]["arow"]
        xh = np.concatenate([results[c]["xhat0"], results[c]["xhat1"]],
                            axis=0)
        X[rows, :] = xh.T
    return A, X


def kernel(x, edge_index, We1, be1, We2, be2, Wa1, ba1, Wa2, ba2, Ws1, bs1,
           trace=False):
    pre = _preprocess(edge_index, N)
    in_maps, pre = make_in_maps(x, edge_index, We1, be1, We2, be2, Wa1, ba1,
                                Wa2, ba2, Ws1, bs1, pre=pre, n=N)
    nc = _get_program(N, pre["Ls"])
    br = run_bass_kernel_spmd(nc, in_maps, list(range(NCORES)), trace=trace)
    A, X = assemble(br.results, pre, N)
    if trace:
        kernel.last_result = br
    return (A, X)


# revision 16
# speedup vs baseline: 272.3021x; 1.0138x over previous
"""Trainium2 Bass kernel for the DOMINANT-style GCN autoencoder.

kernel(**inputs) takes the FULL inputs (x [N,256], edge_index [2,E], weights)
and returns (A_hat [N,N], X_hat [N,256]) exactly like the reference.

Strategy (8 NeuronCores, SPMD single launch):
  - Nodes sharded by row range across the 8 cores (2048 nodes/core), with a
    per-shard degree-sort permutation so gather batches pad tightly.
  - gcn(z,W,b) == relu(P(z) @ W + b)   with   P(u) = dinv*(A@(u*dinv) + u*dinv)
    so conv3 (Wa1) and conv5 (Ws1) share one aggregation: 4 aggregations total.
  - Each aggregation: scale rows by dinv -> AllGather [N,128] into shared DRAM
    -> dma_gather of edge sources (dst-bucketed, padded per 128-node batch)
    -> strided tensor_reduce segment sums -> + self term -> *dinv.
  - A_hat = s @ s.T: s all-gathered, regathered into true node order,
    PE-transposed into sT [128, N] held in SBUF; row-shard matmuls stream
    [128,512] PSUM tiles through DVE/ACT copies out to HBM (128 MiB/core).
"""

import os
import sys

for _p in ("/opt/trn_rl_repo", "/root/.axon_site/_ro/trn_rl_repo"):
    if os.path.isdir(_p) and _p not in sys.path:
        sys.path.insert(0, _p)

import numpy as np

import concourse.bacc as bacc
import concourse.bass as bass
import concourse.mybir as mybir
import concourse.tile as tile
from concourse.bass_utils import run_bass_kernel_spmd
from concourse.library_config import mlp

F32 = mybir.dt.float32
I16 = mybir.dt.int16

NCORES = 8
N = 16384
E = 524288
NFEAT = 256
NHID = 128
P = 128  # partitions


# --------------------------------------------------------------------------
# Host-side preprocessing
# --------------------------------------------------------------------------

def _preprocess(edge_index, n=N):
    """Degree/permutation/gather-index computation. O(N+E) host work."""
    src = np.asarray(edge_index[0], dtype=np.int64)
    dst = np.asarray(edge_index[1], dtype=np.int64)
    s = n // NCORES
    nb = s // P  # batches of 128 nodes per shard

    cnt = np.bincount(dst, minlength=n)  # edge in-degree (no self loop)
    deg = cnt.astype(np.float32) + 1.0
    dinv = (1.0 / np.sqrt(deg)).astype(np.float32)

    # CSR of edges sorted by dst
    order = np.argsort(dst, kind="stable")
    src_s = src[order]
    starts = np.zeros(n + 1, np.int64)
    np.cumsum(cnt, out=starts[1:])

    # per-shard degree sort -> row2node / node2row
    row2node = np.empty(n, np.int64)
    node2row = np.empty(n, np.int64)
    for c in range(NCORES):
        nodes = np.arange(c * s, (c + 1) * s)
        o = np.argsort(cnt[nodes], kind="stable")
        rn = nodes[o]
        row2node[c * s:(c + 1) * s] = rn
        node2row[rn] = np.arange(c * s, (c + 1) * s)

    # uniform (across cores) padded batch lengths
    Ls = []
    for b in range(nb):
        m = 1
        for c in range(NCORES):
            rows = row2node[c * s + b * P: c * s + (b + 1) * P]
            m = max(m, int(cnt[rows].max()))
        Ls.append(m)

    zrow = n  # index of the zeros row appended to every gather source

    def wrap16(flat):
        # device layout: index i lives at [partition i%16, col i//16],
        # replicated across the 8 Q7 core groups -> [128, len/16]
        w = flat.reshape(-1, 16).T
        return np.tile(w, (8, 1)).astype(np.int16)

    idx_inputs = []
    for c in range(NCORES):
        blocks = []
        for b in range(nb):
            L = Ls[b]
            arr = np.full((L, P), zrow, np.int64)  # [col, partition]
            rows = row2node[c * s + b * P: c * s + (b + 1) * P]
            for p in range(P):
                v = rows[p]
                s0, s1 = starts[v], starts[v + 1]
                if s1 > s0:
                    arr[: s1 - s0, p] = node2row[src_s[s0:s1]]
            blocks.append(wrap16(arr.reshape(-1)))
        idx_inputs.append(np.concatenate(blocks, axis=1))

    sidx = wrap16(node2row.copy())  # true node order -> permuted global row

    dinv_sb = []
    for c in range(NCORES):
        d = dinv[row2node[c * s:(c + 1) * s]].reshape(nb, P).T  # [128, nb]
        dinv_sb.append(np.ascontiguousarray(d.astype(np.float32)))

    return dict(
        s=s, nb=nb, Ls=Ls, row2node=row2node, node2row=node2row,
        idx_inputs=idx_inputs, sidx=sidx, dinv_sb=dinv_sb,
    )


# --------------------------------------------------------------------------
# Device program
# --------------------------------------------------------------------------

def build_program(n, Ls, f32r_ahat=False, stop_stage=None, repeat=1):
    """Build the SPMD Bass program. All cores run the same program; per-core
    behaviour differs only through input data (idx tables, x shard, dinv).
    stop_stage truncates the program early (hang bisection)."""
    s = n // NCORES
    nb = s // P
    njc = n // 512  # A_hat column chunks of 512
    nc = bacc.Bacc("TRN2", target_bir_lowering=False, debug=False,
                   num_devices=NCORES)
    rg = [list(range(NCORES))]
    idx_cols = sum(8 * L for L in Ls)

    # ---- external I/O ----
    ein = lambda name, shape, dt=F32: nc.dram_tensor(name, shape, dt,
                                                     kind="ExternalInput")
    xT0 = ein("xT0", [P, s])
    xT1 = ein("xT1", [P, s])
    We1d = ein("We1", [NFEAT, NHID])
    We2d = ein("We2", [NHID, NHID])
    Wa1d = ein("Wa1", [NHID, NHID])
    Ws1d = ein("Ws1", [NHID, NHID])
    Wa2d = ein("Wa2", [NHID, NFEAT])
    be1d = ein("be1", [P, 1])
    be2d = ein("be2", [P, 1])
    ba1d = ein("ba1", [P, 1])
    bs1d = ein("bs1", [P, 1])
    ba2d = ein("ba2", [P, 2])
    dinvd = ein("dinv", [P, nb])
    idxd = ein("idx", [P, idx_cols], I16)
    sidxd = ein("sidx", [P, n // 16], I16)
    identd = ein("ident", [P, P])

    arow = nc.dram_tensor("arow", [s, n], F32, kind="ExternalOutput")
    xhat0 = nc.dram_tensor("xhat0", [P, s], F32, kind="ExternalOutput")
    xhat1 = nc.dram_tensor("xhat1", [P, s], F32, kind="ExternalOutput")

    # ---- internal DRAM ----
    bounce = [nc.dram_tensor(f"bnc{k}", [s, NHID], F32) for k in range(5)]
    agout = [nc.dram_tensor(f"ag{k}", [n + P, NHID], F32, addr_space="Shared")
             for k in range(5)]

    tc_trace = bool(os.environ.get('KERNEL_TRACE_SIM'))
    with tile.TileContext(nc, trace_sim=tc_trace) as tc:
        with (
            tc.tile_pool(name="const", bufs=1) as cpool,
            tc.tile_pool(name="fm", bufs=1) as fmpool,
            tc.tile_pool(name="pmm", bufs=4, space="PSUM") as pmm,
            tc.tile_pool(name="ptr", bufs=4, space="PSUM") as ptr,
        ):
            nc.gpsimd.load_library(mlp)

            def load_const(pool, dram, shape, tag, dt=F32, src=None):
                t = pool.tile(shape, dt, tag=tag, name=tag)
                nc.sync.dma_start(t[:], dram[:] if src is None else src)
                return t

            we1a = load_const(cpool, We1d, [P, NHID], "we1a", src=We1d[0:P, :])
            we1b = load_const(cpool, We1d, [P, NHID], "we1b",
                              src=We1d[P:NFEAT, :])
            we2 = load_const(cpool, We2d, [P, NHID], "we2")
            wa1 = load_const(cpool, Wa1d, [P, NHID], "wa1")
            ws1 = load_const(cpool, Ws1d, [P, NHID], "ws1")
            wa2 = load_const(cpool, Wa2d, [P, NFEAT], "wa2")
            be1 = load_const(cpool, be1d, [P, 1], "be1")
            be2 = load_const(cpool, be2d, [P, 1], "be2")
            ba1 = load_const(cpool, ba1d, [P, 1], "ba1")
            bs1 = load_const(cpool, bs1d, [P, 1], "bs1")
            ba2 = load_const(cpool, ba2d, [P, 2], "ba2")
            dinv = load_const(cpool, dinvd, [P, nb], "dinv")
            sidx = load_const(cpool, sidxd, [P, n // 16], "sidx", dt=I16)
            ident = load_const(cpool, identd, [P, P], "ident")

            idx_off = np.zeros(nb, np.int64)
            acc = 0
            for b in range(nb):
                idx_off[b] = acc
                acc += 8 * Ls[b]

            mmw = min(512, s)
            relu = mybir.ActivationFunctionType.Relu

            def mm_chunks(lhsT_list, out_cb):
                """out_cb(chunk_j, psum_tile) for lhsT.T @ rhs over column
                chunks of the feature-major rhs [128, s]."""
                for j in range(s // mmw):
                    ps = pmm.tile([P, mmw], F32, tag="mmps", name="ps")
                    for ki, lt in enumerate(lhsT_list):
                        nc.tensor.matmul(
                            ps[:], lt[0], lt[1][:, j * mmw:(j + 1) * mmw],
                            start=(ki == 0), stop=(ki == len(lhsT_list) - 1))
                    out_cb(j, ps)

            def new_fm(tag="fmbuf", bufs=4):
                return fmpool.tile([P, s], F32, tag=tag, bufs=bufs, name=tag)

            # ================= conv phase =================
            def conv_phase(vpool, gp):
                xt0 = load_const(vpool, xT0, [P, s], "xt0")
                xt1 = load_const(vpool, xT1, [P, s], "xt1")
                idxs = load_const(vpool, idxd, [P, idx_cols], "idxs", dt=I16)

                # zero the padding rows of the gather sources
                zt = vpool.tile([P, NHID], F32, tag="zero", name="zt")
                nc.vector.memset(zt[:], 0.0)
                for k in range(5):
                    nc.sync.dma_start(agout[k][n:n + P, :], zt[:])

                def to_nm_scaled(fm_tile):
                    """fm [128, s] -> node-major [128,128] tiles * dinv."""
                    tiles = []
                    for b in range(nb):
                        pt = ptr.tile([P, P], F32, tag="trps", name="pt")
                        nc.tensor.transpose(
                            pt[:], fm_tile[:, b * P:(b + 1) * P], ident[:])
                        u = vpool.tile([P, P], F32, tag="unm", bufs=18,
                                       name="u")
                        nc.scalar.activation(
                            u[:], pt[:], mybir.ActivationFunctionType.Copy,
                            scale=dinv[:, b:b + 1])
                        tiles.append(u)
                    return tiles

                def aggregate(k, u_tiles, stop=None):
                    """P(): AllGather scaled rows, gather + segment sums,
                    add self term, scale by dinv -> node-major q tiles."""
                    for b in range(nb):
                        nc.sync.dma_start(bounce[k][b * P:(b + 1) * P, :],
                                          u_tiles[b][:])
                    nc.gpsimd.collective_compute(
                        "AllGather", mybir.AluOpType.bypass, replica_groups=rg,
                        ins=[bounce[k].ap().opt()],
                        outs=[agout[k][0:n, :].opt()])
                    if stop == "ag":
                        t0 = vpool.tile([P, NHID], F32, tag="agchk",
                                        name="t0")
                        nc.sync.dma_start(t0[:], agout[k][0:P, :])
                        nc.sync.dma_start(xhat0[:, 0:NHID], t0[:])
                        return None
                    q_tiles = []
                    for b in range(nb):
                        L = Ls[b]
                        g = gp.tile([P, L, NHID], F32, tag="gath", bufs=2,
                                    name="g")
                        o = int(idx_off[b])
                        # SWDGE ring holds 1024 descriptors; larger gathers
                        # hang the Q7 -> split into <=8-column (1024-idx)
                        # sub-gathers landing in adjacent column ranges.
                        for c0 in range(0, L, 8):
                            cw = min(8, L - c0)
                            nc.gpsimd.dma_gather(
                                g[:, c0:c0 + cw, :], agout[k].ap(),
                                idxs[:, o + 8 * c0:o + 8 * (c0 + cw)],
                                num_idxs=P * cw, num_idxs_reg=P * cw,
                                elem_size=NHID)
                        red = vpool.tile([P, P], F32, tag="red", bufs=4,
                                         name="red")
                        if stop == "gat":
                            nc.vector.tensor_copy(red[:], g[:, 0, :])
                            nc.sync.dma_start(xhat0[:, b * P:(b + 1) * P],
                                              red[:])
                            continue
                        nc.vector.tensor_reduce(
                            red[:], g.rearrange("p c f -> p f c"),
                            axis=mybir.AxisListType.X, op=mybir.AluOpType.add)
                        nc.vector.tensor_tensor(
                            red[:], red[:], u_tiles[b][:],
                            op=mybir.AluOpType.add)
                        q = vpool.tile([P, P], F32, tag="qnm", bufs=18,
                                       name="q")
                        nc.vector.tensor_scalar_mul(q[:], red[:],
                                                    dinv[:, b:b + 1])
                        q_tiles.append(q)
                    if stop == "gat":
                        return None
                    return q_tiles

                def to_fm(q_tiles, func=None, bias=0.0, tag="fmbuf", bufs=4):
                    """node-major -> fm [128, s] via PE transpose, applying
                    func/bias (per-partition == per-feature) on the way."""
                    fm = new_fm(tag, bufs)
                    f = func or mybir.ActivationFunctionType.Copy
                    for b in range(nb):
                        pt = ptr.tile([P, P], F32, tag="trps", name="pt")
                        nc.tensor.transpose(pt[:], q_tiles[b][:], ident[:])
                        if isinstance(bias, float):
                            nc.scalar.activation(
                                fm[:, b * P:(b + 1) * P], pt[:], f)
                        else:
                            nc.scalar.activation(
                                fm[:, b * P:(b + 1) * P], pt[:], f, bias=bias)
                    return fm

                # conv1: h1 = x @ We1 ; z1 = relu(P(h1) + be1)
                h1f = new_fm()
                mm_chunks([(we1a, xt0), (we1b, xt1)],
                          lambda j, ps: nc.scalar.copy(
                              h1f[:, j * mmw:(j + 1) * mmw], ps[:]))
                if stop_stage == "h1":
                    nc.sync.dma_start(xhat0[:, :], h1f[:])
                    return None
                u1t = to_nm_scaled(h1f)
                if stop_stage == "u1":
                    nc.sync.dma_start(xhat0[:, 0:P], u1t[0][:])
                    return None
                q1 = aggregate(0, u1t,
                               stop={"ag1": "ag", "gat1": "gat"}.get(
                                   stop_stage))
                if q1 is None:
                    return None
                if stop_stage == "q1":
                    nc.sync.dma_start(xhat0[:, 0:P], q1[0][:])
                    return None
                z1f = to_fm(q1, func=relu, bias=be1[:, 0:1])
                if stop_stage == "z1":
                    nc.sync.dma_start(xhat0[:, :], z1f[:])
                    return None

                # conv2: h2 = z1 @ We2 ; z2 = relu(P(h2) + be2)
                h2f = new_fm()
                mm_chunks([(we2, z1f)],
                          lambda j, ps: nc.scalar.copy(
                              h2f[:, j * mmw:(j + 1) * mmw], ps[:]))
                q2 = aggregate(1, to_nm_scaled(h2f))
                z2f = to_fm(q2, func=relu, bias=be2[:, 0:1])
                if stop_stage == "conv2":
                    nc.sync.dma_start(xhat0[:, :], z2f[:])
                    return None

                # shared aggregation for conv3 (Wa1) and conv5 (Ws1)
                q3 = aggregate(2, to_nm_scaled(z2f))
                q3f = to_fm(q3)

                # a = relu(q3 @ Wa1 + ba1); s = relu(q3 @ Ws1 + bs1)
                af = new_fm()
                mm_chunks([(wa1, q3f)],
                          lambda j, ps: nc.scalar.activation(
                              af[:, j * mmw:(j + 1) * mmw], ps[:], relu,
                              bias=ba1[:, 0:1]))
                sf = new_fm(tag="sf", bufs=1)
                mm_chunks([(ws1, q3f)],
                          lambda j, ps: nc.scalar.activation(
                              sf[:, j * mmw:(j + 1) * mmw], ps[:], relu,
                              bias=bs1[:, 0:1]))
                if stop_stage == "conv3":
                    nc.sync.dma_start(xhat0[:, :], af[:])
                    nc.sync.dma_start(xhat1[:, :], sf[:])
                    return None

                # conv5 output s first: its AllGather + the downstream
                # regather/transposes are off conv4's critical path, so
                # emitting them here lets them overlap conv4's aggregation.
                for b in range(nb):
                    pt = ptr.tile([P, P], F32, tag="trps", name="pt")
                    nc.tensor.transpose(pt[:], sf[:, b * P:(b + 1) * P],
                                        ident[:])
                    snm = vpool.tile([P, P], F32, tag="snm", bufs=4,
                                     name="snm")
                    nc.scalar.copy(snm[:], pt[:])
                    nc.sync.dma_start(bounce[4][b * P:(b + 1) * P, :],
                                      snm[:])
                nc.gpsimd.collective_compute(
                    "AllGather", mybir.AluOpType.bypass, replica_groups=rg,
                    ins=[bounce[4].ap().opt()],
                    outs=[agout[4][0:n, :].opt()])

                # conv4: X_hat = relu(P(a) @ Wa2 + ba2)
                q4 = aggregate(3, to_nm_scaled(af))
                q4f = to_fm(q4)
                for h, xdram in enumerate((xhat0, xhat1)):
                    xf = new_fm()
                    mm_chunks([(wa2[:, h * P:(h + 1) * P], q4f)],
                              lambda j, ps, xf=xf, h=h: nc.scalar.activation(
                                  xf[:, j * mmw:(j + 1) * mmw], ps[:], relu,
                                  bias=ba2[:, h:h + 1]))
                    nc.sync.dma_start(xdram[:, :], xf[:])
                return sf

            for _rep in range(repeat):
              with (
                tc.tile_pool(name=f"convp{_rep}", bufs=1) as vpool,
                tc.tile_pool(name=f"gat{_rep}", bufs=1) as gp,
              ):
                sf = conv_phase(vpool, gp)

              # ================= A_hat phase =================
              if sf is not None and stop_stage != "noahat":
                with tc.tile_pool(name=f"ahat{_rep}", bufs=1) as apool:
                    # regather s into true node order; transpose -> sT [128,n]
                    sTdt = mybir.dt.float32r if f32r_ahat else F32
                    sT = apool.tile([P, n], sTdt, tag="sT", name="sT")
                    rch = min(1024, n)
                    for r in range(n // rch):
                        rb = apool.tile([P, rch // P, NHID], F32, tag="rgath",
                                        bufs=2, name="rb")
                        nc.gpsimd.dma_gather(
                            rb[:], agout[4].ap(),
                            sidx[:, r * (rch // 16):(r + 1) * (rch // 16)],
                            num_idxs=rch, num_idxs_reg=rch, elem_size=NHID)
                        for cth in range(rch // P):
                            jcol = r * (rch // P) + cth
                            pt = ptr.tile([P, P], F32, tag="trps", name="pt")
                            nc.tensor.transpose(pt[:], rb[:, cth, :],
                                                ident[:])
                            nc.scalar.copy(sT[:, jcol * P:(jcol + 1) * P],
                                           pt[:])

                    # arow = s_shard @ s.T
                    if f32r_ahat:
                        # rounded copy of s for the f32r matmul (the PE f32r
                        # path needs f32r-rounded producers)
                        sfr = apool.tile([P, s], mybir.dt.float32r,
                                         tag="sfr", name="sfr")
                        nc.scalar.copy(sfr[:], sf[:])
                        sfm_mm, sT_mm = sfr, sT
                    else:
                        sfm_mm, sT_mm = sf, sT
                    for i in range(nb):
                        lhsT = sfm_mm[:, i * P:(i + 1) * P]
                        for jo in range(njc // 4):
                            stg = apool.tile([P, 2048], F32, tag="astg",
                                             bufs=4, name="stg")
                            for ji in range(4):
                                j = jo * 4 + ji
                                ps = pmm.tile([P, 512], F32, tag="mmps",
                                              name="ps")
                                nc.tensor.matmul(
                                    ps[:], lhsT,
                                    sT_mm[:, j * 512:(j + 1) * 512],
                                    start=True, stop=True)
                                dst = stg[:, ji * 512:(ji + 1) * 512]
                                if ji % 2 == 0:
                                    nc.vector.tensor_copy(dst, ps[:])
                                else:
                                    nc.scalar.copy(dst, ps[:])
                            nc.sync.dma_start(
                                arow[i * P:(i + 1) * P,
                                     jo * 2048:(jo + 1) * 2048],
                                stg[:])

    nc.compile()
    if tc_trace and getattr(tc, "_perfetto_entries", None):
        # entries: (name, start_ns, end_ns, space, size, addr, tag)
        global LAST_MAKESPAN_NS
        ends = [t[2] for t in tc._perfetto_entries]
        starts = [t[1] for t in tc._perfetto_entries]
        LAST_MAKESPAN_NS = int(max(ends) - min(starts))
        print(f"[cost-model makespan] {LAST_MAKESPAN_NS} ns")
    return nc


# --------------------------------------------------------------------------
# Host entry point
# --------------------------------------------------------------------------

_PROGRAM_CACHE = {}
LAST_MAKESPAN_NS = None


def _get_program(n, Ls, f32r_ahat=False):
    key = (n, tuple(Ls), f32r_ahat)
    if key not in _PROGRAM_CACHE:
        _PROGRAM_CACHE[key] = build_program(n, Ls, f32r_ahat)
    return _PROGRAM_CACHE[key]


def make_in_maps(x, edge_index, We1, be1, We2, be2, Wa1, ba1, Wa2, ba2,
                 Ws1, bs1, pre=None, n=N):
    s = n // NCORES
    if pre is None:
        pre = _preprocess(edge_index, n)
    x = np.asarray(x, np.float32)
    in_maps = []
    shared = {
        "We1": np.asarray(We1, np.float32),
        "We2": np.asarray(We2, np.float32),
        "Wa1": np.asarray(Wa1, np.float32),
        "Ws1": np.asarray(Ws1, np.float32),
        "Wa2": np.asarray(Wa2, np.float32),
        "be1": np.asarray(be1, np.float32).reshape(P, 1),
        "be2": np.asarray(be2, np.float32).reshape(P, 1),
        "ba1": np.asarray(ba1, np.float32).reshape(P, 1),
        "bs1": np.asarray(bs1, np.float32).reshape(P, 1),
        "ba2": np.ascontiguousarray(
            np.asarray(ba2, np.float32).reshape(2, P).T),
        "sidx": pre["sidx"],
        "ident": np.eye(P, dtype=np.float32),
    }
    for c in range(NCORES):
        rows = pre["row2node"][c * s:(c + 1) * s]
        xs = np.ascontiguousarray(x[rows].T)  # [NFEAT, s]
        m = dict(shared)
        m["xT0"] = np.ascontiguousarray(xs[0:P])
        m["xT1"] = np.ascontiguousarray(xs[P:NFEAT])
        m["idx"] = pre["idx_inputs"][c]
        m["dinv"] = pre["dinv_sb"][c]
        in_maps.append(m)
    return in_maps, pre


def assemble(results, pre, n=N):
    s = n // NCORES
    A = np.empty((n, n), np.float32)
    X = np.empty((n, NFEAT), np.float32)
    for c in range(NCORES):
        rows = pre["row2node"][c * s:(c + 1) * s]
        A[rows, :] = results[c]["arow"]
        xh = np.concatenate([results[c]["xhat0"], results[c]["xhat1"]],
                            axis=0)
        X[rows, :] = xh.T
    return A, X


def kernel(x, edge_index, We1, be1, We2, be2, Wa1, ba1, Wa2, ba2, Ws1, bs1,
           trace=False):
    pre = _preprocess(edge_index, N)
    in_maps, pre = make_in_maps(x, edge_index, We1, be1, We2, be2, Wa1, ba1,
                                Wa2, ba2, Ws1, bs1, pre=pre, n=N)
    nc = _get_program(N, pre["Ls"])
    br = run_bass_kernel_spmd(nc, in_maps, list(range(NCORES)), trace=trace)
    A, X = assemble(br.results, pre, N)
    if trace:
        kernel.last_result = br
    return (A, X)
